# revision 1
# baseline (speedup 1.0000x reference)
"""Causal self-attention with interleaved RoPE, tensor-parallel over heads on 8 NeuronCores.

Strategy (per core c, heads hA=2c, hB=2c+1):
  - All on-chip tensors live "transposed": feature dim on partitions, tokens on free dim.
  - QKV projection: psum[dcol, tok] = qkv_wT_tile.T @ xT_tile  (contraction over C in 8 tiles).
  - RoPE applied in-transposed layout: q_rot = q*cosT + swap(q*sinTt) where swap
    (adjacent-partition exchange) is a DVE stream_shuffle and the sin table is
    sign-folded + pair-reindexed on the host so the swap commutes.
  - Scores computed transposed: S^T[tk, tq] = K^T.T @ Q^T per head, two heads packed in
    the PE array via row tiling (contraction = Dh = 64 each).
  - softmax: exp on ACT (scale=1/8 folded in; key-mask bias per partition folded in);
    causal masking of diagonal blocks via host-precomputed mask multiply; row sums come
    free from an all-ones block col-packed into the AV matmul; normalization =
    reciprocal + multiply on DVE.
  - AV: y'^T[{d|r}, tq] += [V_h | ones].T @ E^T per tk tile (V transposed on PE;
    the ones half-block makes rows 64:128 the softmax denominator, pre-broadcast).
  - Out-projection: partial out^T[c_out, tq] = owT.T @ y_norm^T, DMA'd psum->HBM.
  - Host: gathers 8 partial outputs, sums, applies query mask and out bias.
Matmuls use float32r (1 cycle/row at N>=512): tensors feeding matmuls are declared
float32r end-to-end; host pre-rounds DMA'd data to the 11-mantissa-bit format.
"""

import numpy as np

B, T, C = 2, 2048, 1024
H, DH = 16, 64
NCORES = 8
CT = C // 128  # 8 contraction tiles
NEG = -1e30

_PROGRAM_CACHE = {}
LAST_RESULTS = None


def _build_program(has_qkv_bias=False):
    import concourse.mybir as mybir
    import concourse.tile as tile
    from concourse import bacc
    from contextlib import ExitStack

    F32 = mybir.dt.float32
    F32R = mybir.dt.float32r
    EXP = mybir.ActivationFunctionType.Exp

    SWAP_MASK = [i ^ 1 for i in range(32)]
    nc = bacc.Bacc("TRN2", target_bir_lowering=False, debug=False)

    # ---- DRAM I/O ----
    xT_d = nc.dram_tensor("xT", (CT, 128, B, T), F32R, kind="ExternalInput")
    qkvwT_d = nc.dram_tensor("qkvwT", (3, CT, 128, 128), F32R, kind="ExternalInput")
    bqkv_d = nc.dram_tensor("bqkv", (128, 3), F32, kind="ExternalInput")
    owT_d = nc.dram_tensor("owT", (8, 128, 128), F32R, kind="ExternalInput")
    cosT_d = nc.dram_tensor("cosT", (128, T), F32, kind="ExternalInput")
    sinTt_d = nc.dram_tensor("sinTt", (128, T), F32, kind="ExternalInput")
    triC_d = nc.dram_tensor("triC", (128, 2048), F32, kind="ExternalInput")
    vones_d = nc.dram_tensor("vones", (128, 256), F32R, kind="ExternalInput")
    ident_d = nc.dram_tensor("ident", (128, 128), F32, kind="ExternalInput")
    expb_d = nc.dram_tensor("expb", (128, 2 * (T // 128)), F32, kind="ExternalInput")
    outp_d = nc.dram_tensor("outp", (8, 128, B, T), F32, kind="ExternalOutput")

    NTK = T // 128  # 16 key tiles
    NJ = T // 512  # 4 query tiles

    with tile.TileContext(nc) as tc, ExitStack() as ctx:
        cpool = ctx.enter_context(tc.tile_pool(name="consts", bufs=1))
        xpool = ctx.enter_context(tc.tile_pool(name="xt", bufs=CT))
        spool = ctx.enter_context(tc.tile_pool(name="seq", bufs=2))
        vpool = ctx.enter_context(tc.tile_pool(name="vsb", bufs=20))
        epool = ctx.enter_context(tc.tile_pool(name="eexp", bufs=6))
        tpool = ctx.enter_context(tc.tile_pool(name="tmp", bufs=2))
        rpool = ctx.enter_context(tc.tile_pool(name="rr", bufs=2))
        spsum = ctx.enter_context(tc.tile_pool(name="S", bufs=2, space="PSUM"))
        qpool = ctx.enter_context(tc.tile_pool(name="qp", bufs=2, space="PSUM"))
        ypool = ctx.enter_context(tc.tile_pool(name="yp", bufs=2, space="PSUM"))

        def load_const(nm, dram_ap, shape, dt=F32):
            t = cpool.tile(shape, dt, name=nm, tag=nm)
            nc.sync.dma_start(t[:], dram_ap)
            return t

        qkvw = [
            [
                load_const(f"c_w{s}_{k}", qkvwT_d[s, k, :, :], [128, 128], F32R)
                for k in range(CT)
            ]
            for s in range(3)
        ]

        # dummy exp so the ACT table set loads during the initial DMA fill
        # instead of on the first real softmax tile
        warm = cpool.tile([128, 1], F32, name="warm", tag="warm")
        nc.vector.memset(warm[:], 0.0)
        nc.scalar.activation(warm[:], warm[:], EXP)

        def load_xt_half(b, half):
            xt = []
            for k in range(CT):
                t = xpool.tile([128, T // 2], F32R, tag="xt", name=f"xt{b}_{half}_{k}")
                nc.sync.dma_start(
                    t[:], xT_d[k, :, b, 1024 * half : 1024 * (half + 1)]
                )
                xt.append(t)
            return xt

        # DMA issue order = need order: first strip, rope tables, second strip,
        # transpose/attention consts, out weights, batch-1 strips.
        xts = {}
        xts[(0, 0)] = load_xt_half(0, 0)
        cosT = load_const("c_cos", cosT_d[:, :], [128, T])
        sinTt = load_const("c_sin", sinTt_d[:, :], [128, T])
        bqkv = load_const("c_bq", bqkv_d[:, :], [128, 3])
        ident = load_const("c_id", ident_d[:, :], [128, 128])
        vones = load_const("c_ones", vones_d[:, :], [128, 256], F32R)
        triC = load_const("c_tri", triC_d[:, :], [128, 2048])
        expb = load_const("c_eb", expb_d[:, :], [128, 2 * NTK])
        xts[(0, 1)] = load_xt_half(0, 1)
        owT = [
            load_const(f"c_ow{m}", owT_d[m, :, :], [128, 128], F32R) for m in range(8)
        ]
        xts[(1, 0)] = load_xt_half(1, 0)
        xts[(1, 1)] = load_xt_half(1, 1)

        for b in range(B):
            q2T = spool.tile([128, T], F32R, tag="q2T")
            k2T = spool.tile([128, T], F32R, tag="k2T")
            v2T = spool.tile([128, T], F32, tag="v2T")
            dsts = [q2T, k2T, v2T]

            # ---- QKV projection + RoPE, query-chunk-major so attention can
            # start as soon as the first (q,k,v) triple lands ----
            vsb = []
            for half in range(2):
                xt = xts[(b, half)]
                for jh in range(2):
                    jc = 2 * half + jh
                    sl = slice(512 * jc, 512 * (jc + 1))
                    xsl = slice(512 * jh, 512 * (jh + 1))
                    for s in range(3):
                        ps = qpool.tile([128, 512], F32, tag="qp")
                        for k in range(CT):
                            nc.tensor.matmul(
                                ps[:],
                                qkvw[s][k][:],
                                xt[k][:, xsl],
                                start=(k == 0),
                                stop=(k == CT - 1),
                            )
                        if has_qkv_bias:
                            nc.vector.tensor_scalar_add(
                                ps[:], ps[:], bqkv[:, s : s + 1]
                            )
                        if s == 2:
                            nc.vector.tensor_copy(v2T[:, sl], ps[:])
                        else:
                            t1 = tpool.tile([128, 512], F32, tag="t1")
                            t2 = tpool.tile([128, 512], F32, tag="t2")
                            t2s = tpool.tile([128, 512], F32, tag="t2s")
                            nc.vector.tensor_mul(t1[:], ps[:], cosT[:, sl])
                            nc.vector.tensor_mul(t2[:], ps[:], sinTt[:, sl])
                            nc.vector.stream_shuffle(t2s[:], t2[:], SWAP_MASK)
                            nc.gpsimd.tensor_add(dsts[s][:, sl], t1[:], t2s[:])
                    # transpose this chunk's V tiles: 4 PE transposes into one
                    # psum slot, then unpack into [V_A | 1s | V_B | 1s] tiles
                    vtg = qpool.tile([128, 512], F32, tag="qp", name=f"vtg{b}_{jc}")
                    for u in range(4):
                        t = 4 * jc + u
                        nc.tensor.transpose(
                            vtg[:, 128 * u : 128 * (u + 1)],
                            v2T[:, 128 * t : 128 * (t + 1)],
                            ident[:],
                        )
                    for u in range(4):
                        t = 4 * jc + u
                        vs = vpool.tile(
                            [128, 256], F32R, tag="vsb", name=f"vs{b}_{t}"
                        )
                        nc.gpsimd.tensor_copy(vs[:, 64:128], vones[:, 64:128])
                        nc.gpsimd.tensor_copy(vs[:, 192:256], vones[:, 192:256])
                        nc.vector.tensor_copy(
                            vs[:, 0:64], vtg[:, 128 * u : 128 * u + 64]
                        )
                        nc.vector.tensor_copy(
                            vs[:, 128:192], vtg[:, 128 * u + 64 : 128 * u + 128]
                        )
                        vsb.append(vs)

            # ---- attention (2 heads packed) ----
            y2T = spool.tile([128, T], F32R, tag="y2T")
            for j in range(NJ):
                jsl = slice(512 * j, 512 * (j + 1))
                yp = [
                    ypool.tile([128, 512], F32, tag="yp", name=f"yp{b}_{j}_{h}")
                    for h in range(2)
                ]
                ntk_j = 4 * (j + 1)
                for t in range(ntk_j):
                    tsl = slice(128 * t, 128 * (t + 1))
                    ecol = b * NTK + t
                    S = spsum.tile([128, 1024], F32, tag="S")
                    for h in range(2):
                        hsl = slice(64 * h, 64 * (h + 1))
                        nc.tensor.matmul(
                            S[:, 512 * h : 512 * (h + 1)],
                            k2T[hsl, tsl],
                            q2T[hsl, jsl],
                            start=True,
                            stop=True,
                            tile_position=(64 * h, 0),
                        )
                    E = epool.tile([128, 1024], F32R, tag="E")
                    m = t - 4 * j if t >= 4 * j else -1
                    if m >= 1:
                        # diagonal tile: skip exp over the fully-masked leading
                        # cols (zeroed explicitly), two-segment AP over both heads
                        for h in range(2):
                            nc.gpsimd.tensor_scalar_mul(
                                E[:, 512 * h : 512 * h + 128 * m],
                                triC[:, 0 : 128 * m],
                                0.0,
                            )
                        seg = E[:, 0:1024].rearrange(
                            "p (h c) -> p h c", h=2
                        )[:, :, 128 * m : 512]
                        sseg = S[:, 0:1024].rearrange(
                            "p (h c) -> p h c", h=2
                        )[:, :, 128 * m : 512]
                        nc.scalar.activation(
                            seg,
                            sseg,
                            EXP,
                            bias=expb[:, ecol : ecol + 1],
                            scale=0.125,
                        )
                    else:
                        nc.scalar.activation(
                            E[:],
                            S[:],
                            EXP,
                            bias=expb[:, ecol : ecol + 1],
                            scale=0.125,
                        )
                    if m >= 0:
                        for h in range(2):
                            nc.gpsimd.tensor_mul(
                                E[:, 512 * h + 128 * m : 512 * h + 128 * (m + 1)],
                                E[:, 512 * h + 128 * m : 512 * h + 128 * (m + 1)],
                                triC[:, 512 * m + 128 * m : 512 * m + 128 * (m + 1)],
                            )
                    last = t == ntk_j - 1
                    for h in range(2):
                        nc.tensor.matmul(
                            yp[h][:],
                            vsb[t][:, 128 * h : 128 * h + 128],
                            E[:, 512 * h : 512 * (h + 1)],
                            start=(t == 0),
                            stop=last,
                        )
                # normalize: y / rowsum, write into stacked y2T
                for h in range(2):
                    hsl = slice(64 * h, 64 * (h + 1))
                    rr = rpool.tile([64, 512], F32, tag="rr")
                    nc.vector.reciprocal(rr[:], yp[h][64:128, :])
                    nc.vector.tensor_mul(y2T[hsl, jsl], yp[h][0:64, :], rr[:])

                # ---- output projection for this query tile (deprioritized
                # so it fills gaps instead of blocking the next j's softmax) ----
                for mt in range(8):
                    op = qpool.tile(
                        [128, 512], F32, tag="qp", name=f"op{b}_{j}_{mt}"
                    )
                    nc.tensor.matmul(
                        op[:],
                        owT[mt][:],
                        y2T[:, jsl],
                        start=True,
                        stop=True,
                    )
                    ot = epool.tile(
                        [128, 512], F32, tag="ot", name=f"ot{b}_{j}_{mt}", bufs=3
                    )
                    if mt % 2 == 0:
                        nc.vector.tensor_copy(ot[:], op[:])
                    else:
                        nc.scalar.copy(ot[:], op[:])
                    nc.scalar.dma_start(outp_d[mt, :, b, jsl], ot[:])


    nc.compile()
    return nc


def _round_fp32r(a):
    """Round-to-nearest-even to fp32r (1s+8e+11m, value kept in the fp32 high bits)."""
    u = np.ascontiguousarray(a, np.float32).view(np.uint32)
    keep = u & np.uint32(0xFFFFF000)
    rem = u & np.uint32(0x00000FFF)
    lsb = (u >> np.uint32(12)) & np.uint32(1)
    up = (rem > 0x800) | ((rem == 0x800) & (lsb == 1))
    return (keep + (up.astype(np.uint32) << np.uint32(12))).view(np.float32)


def _host_inputs(x, attention_mask, qkv_w, qkv_b, out_w):
    """Build the device input tensors. Returns (shared dict, per-core list of dicts)."""
    x = np.ascontiguousarray(np.asarray(x, np.float32))
    qkv_w = np.asarray(qkv_w, np.float32)
    qkv_b = np.asarray(qkv_b, np.float32)
    out_w = np.asarray(out_w, np.float32)
    am = np.asarray(attention_mask)

    xT = _round_fp32r(x.transpose(2, 0, 1).reshape(CT, 128, B, T))

    # RoPE tables (match reference: interleaved rotate, concatenated freq table)
    inv_freq = 1.0 / (10000.0 ** (np.arange(0, DH, 2, dtype=np.float64) / DH))
    tt = np.arange(T, dtype=np.float64)
    freqs = np.outer(tt, inv_freq)  # [T, 32]
    emb = np.concatenate([freqs, freqs], axis=-1)  # [T, 64]
    cos = np.cos(emb).astype(np.float32).T  # [64, T]
    sin = np.sin(emb).astype(np.float32).T  # [64, T]
    sinTt64 = np.empty((DH, T), np.float32)
    sinTt64[0::2] = sin[1::2]  # sinTt[2i]   = +sin[2i+1]
    sinTt64[1::2] = -sin[0::2]  # sinTt[2i+1] = -sin[2i]
    cosT = np.ascontiguousarray(np.tile(cos, (2, 1)))  # [128, T]
    sinTt = np.ascontiguousarray(np.tile(sinTt64, (2, 1)))

    triC = np.zeros((128, 2048), np.float32)
    cc = np.arange(512)[None, :]
    pp = np.arange(128)[:, None]
    for m in range(4):
        triC[:, 512 * m : 512 * (m + 1)] = (cc >= 128 * m + pp).astype(np.float32)

    vones = np.ones((128, 256), np.float32)
    ident = np.eye(128, dtype=np.float32)

    ntk = T // 128
    key_ok = am.astype(bool).reshape(B, ntk, 128)  # [b, t, p]
    expb = np.where(key_ok, 0.0, NEG).astype(np.float32)
    expb = np.ascontiguousarray(expb.transpose(2, 0, 1).reshape(128, B * ntk))

    shared = dict(
        xT=xT, cosT=cosT, sinTt=sinTt, triC=triC,
        vones=vones, ident=ident, expb=expb,
    )

    per_core = []
    for c in range(NCORES):
        r0 = 128 * c
        qkvwT = _round_fp32r(
            np.stack(
                [
                    np.ascontiguousarray(
                        qkv_w[s * C + r0 : s * C + r0 + 128, :].T
                    ).reshape(CT, 128, 128)
                    for s in range(3)
                ]
            )
        )
        bqkv = np.stack(
            [qkv_b[s * C + r0 : s * C + r0 + 128] for s in range(3)], axis=1
        )  # [128, 3]
        ow_slice = out_w[:, r0 : r0 + 128]  # [1024, 128]
        owT = _round_fp32r(ow_slice.reshape(8, 128, 128).transpose(0, 2, 1))
        per_core.append(
            dict(
                qkvwT=qkvwT,
                bqkv=np.ascontiguousarray(bqkv),
                owT=owT,
            )
        )
    return shared, per_core


def kernel(x, attention_mask, qkv_w, qkv_b, out_w, out_b, _trace=False):
    global LAST_RESULTS
    from concourse.bass_utils import run_bass_kernel_spmd

    key = ("nc", bool(np.any(np.asarray(qkv_b))))
    if key not in _PROGRAM_CACHE:
        _PROGRAM_CACHE[key] = _build_program(has_qkv_bias=key[1])
    nc = _PROGRAM_CACHE[key]

    shared, per_core = _host_inputs(x, attention_mask, qkv_w, qkv_b, out_w)
    in_maps = [{**shared, **pc} for pc in per_core]

    res = run_bass_kernel_spmd(
        nc,
        in_maps,
        core_ids=list(range(NCORES)),
        trace=_trace,
        trace_cores=list(range(NCORES)) if _trace else None,
        stitch_traces=bool(_trace),
    )
    LAST_RESULTS = res

    acc = np.zeros((B, T, C), np.float64)
    for c in range(NCORES):
        part = res.results[c]["outp"]  # [8, 128, B, T]
        acc += part.transpose(2, 3, 0, 1).reshape(B, T, C)

    qm = np.asarray(attention_mask).astype(bool)
    out = np.where(qm[..., None], acc, 0.0) + np.asarray(out_b, np.float64)[None, None]
    return out.astype(np.float32)



# revision 14
# speedup vs baseline: 1.4030x; 1.4030x over previous
"""Causal self-attention with interleaved RoPE on 8 NeuronCores.

Sharding: batch x tensor-parallel. Core c handles batch c//4 and heads
4*(c%4) .. 4*(c%4)+3 (two head-pairs hp=0,1). Each core loads only its
batch's activations (bf16), computes QKV + RoPE + attention for its 4
heads, and writes a bf16 partial output [1024, T] (contraction over its
256 head dims); the host sums 4 partials per batch and adds the bias.

Per-core structure (per head-pair hp, packed heads hA, hB):
  - On-chip tensors live "transposed": feature dim on partitions, tokens
    on the free dim. All input/output DMAs are batched into ~20 large
    transfers (DMA issue costs ~0.6-1.3us of sequencer time each).
  - QKV q,k: psum[row, tok] = w_tile.T @ x_tile (contraction over C in 8
    bf16 tiles). RoPE applied in-transposed layout via DVE stream_shuffle
    with a sign-folded, pair-reindexed sin table.
  - V is produced directly token-major: psum[tok, dim] = x_blk.T @ wv_tile
    (stationary = x block, moving = v weights); one copy lands it in the
    persistent AV stationary tile [V_A | V_B | ones] whose ones block
    makes the AV matmul also emit softmax row sums.
  - Scores transposed: S^T[tk, tq] = K^T.T @ Q^T per head, 2 heads packed
    via PE row tiling. Causal masking via subrange matmuls/exp on diagonal
    tiles plus a host tri mask for the partial block. exp on ACT (scale
    1/8 + key-mask bias folded in).
  - hp1's QKV groups are interleaved into hp0's attention tile loop so
    the PE fills the gaps of the ACT(exp)-bound attention phase.
  - Out-projection runs once per query tile during hp=1, accumulating
    both head-pairs (contraction 256) in one psum group; bf16 partials
    leave via gpsimd SWDGE (keeps the ACT sequencer free for exp).
"""

import numpy as np

B, T, C = 2, 2048, 1024
H, DH = 16, 64
NCORES = 8
CT = C // 128  # 8 contraction tiles
NTK = T // 128  # 16 key tiles
NJ = T // 512  # 4 query blocks
NEG = -1e30

_PROGRAM_CACHE = {}
LAST_RESULTS = None


def _build_program(has_qkv_bias=False):
    import concourse.mybir as mybir
    import concourse.tile as tile
    from concourse import bacc
    from contextlib import ExitStack

    F32 = mybir.dt.float32
    F32R = mybir.dt.float32r
    BF16 = mybir.dt.bfloat16
    EXP = mybir.ActivationFunctionType.Exp

    SWAP_MASK = [i ^ 1 for i in range(32)]
    nc = bacc.Bacc("TRN2", target_bir_lowering=False, debug=False)

    # ---- DRAM I/O ----
    xT_d = nc.dram_tensor("xT", (128, CT, T), BF16, kind="ExternalInput")
    qkvwT_d = nc.dram_tensor("qkvwT", (2, 3, 128, CT, 128), BF16, kind="ExternalInput")
    bqkv_d = nc.dram_tensor("bqkv", (2, 128, 4), F32, kind="ExternalInput")
    vbb_d = nc.dram_tensor("vbb", (2, 128, 128), F32, kind="ExternalInput")
    owT_d = nc.dram_tensor("owT", (2, 128, 8, 128), F32R, kind="ExternalInput")
    cosT_d = nc.dram_tensor("cosT", (128, T), BF16, kind="ExternalInput")
    sinTt_d = nc.dram_tensor("sinTt", (128, T), BF16, kind="ExternalInput")
    triC_d = nc.dram_tensor("triC", (128, 128), BF16, kind="ExternalInput")
    expb_d = nc.dram_tensor("expb", (128, NTK), F32, kind="ExternalInput")
    outp_d = nc.dram_tensor("outp", (128, 8, T), BF16, kind="ExternalOutput")

    with tile.TileContext(nc) as tc, ExitStack() as ctx:
        cpool = ctx.enter_context(tc.tile_pool(name="consts", bufs=1))
        spool = ctx.enter_context(tc.tile_pool(name="seq", bufs=2))
        y2pool = ctx.enter_context(tc.tile_pool(name="y2", bufs=1))
        vpool = ctx.enter_context(tc.tile_pool(name="vsb", bufs=1))
        epool = ctx.enter_context(tc.tile_pool(name="eexp", bufs=4))
        opool = ctx.enter_context(tc.tile_pool(name="otp", bufs=2))
        tpool = ctx.enter_context(tc.tile_pool(name="tmp", bufs=2))
        rpool = ctx.enter_context(tc.tile_pool(name="rr", bufs=2))
        spsum = ctx.enter_context(tc.tile_pool(name="S", bufs=2, space="PSUM"))
        qpool = ctx.enter_context(tc.tile_pool(name="qp", bufs=2, space="PSUM"))
        ypool = ctx.enter_context(tc.tile_pool(name="yp", bufs=2, space="PSUM"))

        def load_const(nm, dram_ap, shape, dt=F32):
            t = cpool.tile(shape, dt, name=nm, tag=nm)
            nc.sync.dma_start(t[:], dram_ap)
            return t

        # ---- batched DMAs, issue order = need order ----
        # per (hp, s): one [128, CT*128] tile, k-th stationary at cols 128k
        qw = {}
        qw[(0, 0)] = load_const("w00", qkvwT_d[0, 0], [128, CT * 128], BF16)
        qw[(0, 1)] = load_const("w01", qkvwT_d[0, 1], [128, CT * 128], BF16)
        # x quarter q: [128, CT*512], k-th tile's 512 tokens at cols 512k
        xq = []
        for q in range(4):
            t = cpool.tile([128, CT * 512], BF16, name=f"xq{q}", tag=f"xq{q}")
            nc.sync.dma_start(t[:], xT_d[:, :, 512 * q : 512 * (q + 1)])
            xq.append(t)
        cosT = load_const("c_cos", cosT_d[:, :], [128, T], BF16)
        sinTt = load_const("c_sin", sinTt_d[:, :], [128, T], BF16)
        qw[(0, 2)] = load_const("w02", qkvwT_d[0, 2], [128, CT * 128], BF16)
        triC = load_const("c_tri", triC_d[:, :], [128, 128], BF16)
        expb = load_const("c_eb", expb_d[:, :], [128, NTK])
        qw[(1, 0)] = load_const("w10", qkvwT_d[1, 0], [128, CT * 128], BF16)
        qw[(1, 1)] = load_const("w11", qkvwT_d[1, 1], [128, CT * 128], BF16)
        qw[(1, 2)] = load_const("w12", qkvwT_d[1, 2], [128, CT * 128], BF16)
        ow = [
            load_const(f"ow{hp}", owT_d[hp], [128, 8 * 128], F32R) for hp in range(2)
        ]
        if has_qkv_bias:
            bqkv = [
                load_const(f"c_bq{hp}", bqkv_d[hp], [128, 4]) for hp in range(2)
            ]
            vbb = [
                load_const(f"c_vb{hp}", vbb_d[hp], [128, 128]) for hp in range(2)
            ]

        # dummy exp so the ACT table set loads during the initial DMA fill
        warm = cpool.tile([128, 1], F32, name="warm", tag="warm")
        nc.vector.memset(warm[:], 0.0)
        nc.scalar.activation(warm[:], warm[:], EXP)

        # persistent AV stationary tiles [V_A | ones | V_B | ones]; the ones
        # halves (written once) make the AV matmul emit softmax row sums
        onesrc = cpool.tile([128, 64], F32, name="onesrc", tag="onesrc")
        nc.vector.memset(onesrc[:], 1.0)
        vsb = {}
        for hp in range(2):
            for t in range(NTK):
                vs = vpool.tile([128, 256], F32R, tag=f"vs{hp}_{t}", name=f"vs{hp}_{t}")
                nc.vector.tensor_copy(vs[:, 64:128], onesrc[:])
                nc.vector.tensor_copy(vs[:, 192:256], onesrc[:])
                vsb[(hp, t)] = vs

        y2T = {}
        qk2T = {}

        def qkv_stage(hp):
            q2T = spool.tile([128, T], F32R, tag="q2T", name=f"q2T{hp}")
            k2T = spool.tile([128, T], F32R, tag="k2T", name=f"k2T{hp}")
            qk2T[hp] = (q2T, k2T)
            dsts = [q2T, k2T]
            for jc in range(NJ):
                sl = slice(512 * jc, 512 * (jc + 1))
                for s in range(2):
                    ps = qpool.tile([128, 512], F32, tag="qp", name=f"ps{hp}_{jc}_{s}")
                    for k in range(CT):
                        nc.tensor.matmul(
                            ps[:],
                            qw[(hp, s)][:, 128 * k : 128 * (k + 1)],
                            xq[jc][:, 512 * k : 512 * (k + 1)],
                            start=(k == 0),
                            stop=(k == CT - 1),
                        )
                    if has_qkv_bias:
                        nc.vector.tensor_scalar_add(
                            ps[:], ps[:], bqkv[hp][:, s : s + 1]
                        )
                    t1 = tpool.tile([128, 512], F32, tag="t1", name=f"t1_{hp}_{jc}_{s}")
                    t2 = tpool.tile([128, 512], F32, tag="t2", name=f"t2_{hp}_{jc}_{s}")
                    t2s = tpool.tile(
                        [128, 512], F32, tag="t2s", name=f"t2s_{hp}_{jc}_{s}"
                    )
                    nc.vector.tensor_mul(t1[:], ps[:], cosT[:, sl])
                    nc.vector.tensor_mul(t2[:], ps[:], sinTt[:, sl])
                    nc.vector.stream_shuffle(t2s[:], t2[:], SWAP_MASK)
                    nc.gpsimd.tensor_add(dsts[s][:, sl], t1[:], t2s[:])
                    yield
                # V token-major: 4 tiles of [128 tok, 128 dim] per chunk
                vt = qpool.tile([128, 512], F32, tag="qp", name=f"vt{hp}_{jc}")
                for u in range(4):
                    for k in range(CT):
                        nc.tensor.matmul(
                            vt[:, 128 * u : 128 * (u + 1)],
                            xq[jc][:, 512 * k + 128 * u : 512 * k + 128 * (u + 1)],
                            qw[(hp, 2)][:, 128 * k : 128 * (k + 1)],
                            start=(k == 0),
                            stop=(k == CT - 1),
                        )
                for u in range(4):
                    tki = 4 * jc + u
                    vs = vsb[(hp, tki)]
                    vtu = vt[:, 128 * u : 128 * (u + 1)]
                    vdst = vs[:, 0:192].rearrange("p (a b) -> p a b", b=64)[
                        :, 0::2, :
                    ]
                    if has_qkv_bias:
                        nc.vector.tensor_add(vdst, vtu, vbb[hp][:])
                    else:
                        nc.vector.tensor_copy(vdst, vtu)
                yield

        def out_proj_units(j):
            """Output projection for query block j: one unit per mt slice
            (2 accumulating matmuls + bf16 copy), plus 2 half DMAs."""
            jsl = slice(512 * j, 512 * (j + 1))
            otj = opool.tile([128, 8 * 512], BF16, tag="ot", name=f"ot{j}")
            for mt in range(8):
                def unit(mt=mt):
                    op = qpool.tile([128, 512], F32, tag="qp", name=f"op{j}_{mt}")
                    nc.tensor.matmul(
                        op[:], ow[0][:, 128 * mt : 128 * (mt + 1)],
                        y2T[0][:, jsl], start=True, stop=False,
                    )
                    nc.tensor.matmul(
                        op[:], ow[1][:, 128 * mt : 128 * (mt + 1)],
                        y2T[1][:, jsl], start=False, stop=True,
                    )
                    osl = slice(512 * mt, 512 * (mt + 1))
                    if mt % 2 == 0:
                        nc.vector.tensor_copy(otj[:, osl], op[:])
                    else:
                        nc.scalar.copy(otj[:, osl], op[:])
                    if mt == 3:
                        nc.gpsimd.dma_start(
                            outp_d[:, 0:4, jsl], otj[:, 0 : 4 * 512]
                        )
                    elif mt == 7:
                        nc.gpsimd.dma_start(
                            outp_d[:, 4:8, jsl], otj[:, 4 * 512 : 8 * 512]
                        )
                yield unit

        def attention_stage(hp, jorder, feed, feed_rate, do_out):
            """Software-pipelined attention: AV(t) is emitted one tile late so
            S(t+1) sits ahead of it in the in-order PE queue while exp(t)
            runs. `feed` is a deque of callables (hp1 QKV units / out-proj
            units) drained between tiles to fill PE gaps."""
            q2T, k2T = qk2T[hp]
            pending = [None]

            def flush():
                if pending[0] is not None:
                    av, posts = pending[0]
                    pending[0] = None
                    av()
                    for p in posts:
                        p()

            for j in jorder:
                jsl = slice(512 * j, 512 * (j + 1))
                yp = [
                    ypool.tile([128, 512], F32, tag="yp", name=f"yp{hp}_{j}_{h}")
                    for h in range(2)
                ]
                ntk_j = 4 * (j + 1)
                for t in range(ntk_j):
                    tsl = slice(128 * t, 128 * (t + 1))
                    m = t - 4 * j if t >= 4 * j else -1
                    # diagonal tile m: query cols [0, 128m) see no valid keys
                    # in this tile; restrict S/exp/AV to cols [128m, 512).
                    # (m<=0 or S-moving<256 cols would hit the fp32r slow
                    # path, so only trim S for m in {1,2}.)
                    ms = 128 * m if m in (1, 2) else 0
                    S = spsum.tile([128, 1024], F32, tag="S")
                    for h in range(2):
                        hsl = slice(64 * h, 64 * (h + 1))
                        nc.tensor.matmul(
                            S[:, 512 * h + ms : 512 * (h + 1)],
                            k2T[hsl, tsl],
                            q2T[hsl, 512 * j + ms : 512 * (j + 1)],
                            start=True,
                            stop=True,
                            tile_position=(64 * h, 0),
                        )
                    E = epool.tile([128, 1024], F32R, tag="E")
                    if m >= 1:
                        seg = E[:, 0:1024].rearrange("p (h c) -> p h c", h=2)[
                            :, :, 128 * m : 512
                        ]
                        sseg = S[:, 0:1024].rearrange("p (h c) -> p h c", h=2)[
                            :, :, 128 * m : 512
                        ]
                        nc.scalar.activation(
                            seg, sseg, EXP, bias=expb[:, t : t + 1], scale=0.125
                        )
                    else:
                        nc.scalar.activation(
                            E[:], S[:], EXP, bias=expb[:, t : t + 1], scale=0.125
                        )
                    if m >= 0:
                        for h in range(2):
                            nc.gpsimd.tensor_mul(
                                E[:, 512 * h + 128 * m : 512 * h + 128 * (m + 1)],
                                E[:, 512 * h + 128 * m : 512 * h + 128 * (m + 1)],
                                triC[:, 0:128],
                            )
                    flush()

                    def mk_av(j=j, t=t, m=m, E=E, yp=yp, last=(t == ntk_j - 1)):
                        ma = 128 * m if m >= 1 else 0
                        for h in range(2):
                            nc.tensor.matmul(
                                yp[h][:, ma:512],
                                vsb[(hp, t)][:, 128 * h : 128 * (h + 1)],
                                E[:, 512 * h + ma : 512 * (h + 1)],
                                start=(t == 0),
                                stop=last,
                                skip_group_check=True,
                            )

                    posts = []
                    if t == ntk_j - 1:

                        def normalize(j=j, jsl=jsl, yp=yp):
                            for h in range(2):
                                hsl = slice(64 * h, 64 * (h + 1))
                                rr = rpool.tile(
                                    [64, 512], F32, tag="rr", name=f"rr{hp}_{j}_{h}"
                                )
                                nc.vector.reciprocal(rr[:], yp[h][64:128, :])
                                nc.vector.tensor_mul(
                                    y2T[hp][hsl, jsl], yp[h][0:64, :], rr[:]
                                )
                            if do_out:
                                feed.extend(out_proj_units(j))

                        posts.append(normalize)
                    pending[0] = (mk_av, posts)
                    for _ in range(feed_rate):
                        if feed:
                            feed.popleft()()
                    yield
            flush()
            while feed:
                feed.popleft()()

        # ---- schedule: QKV(0); attn(0) with QKV(1) interleaved; attn(1)
        # with out-proj interleaved (j order puts the longest block first
        # after j=0 so its out-proj units fill later attention blocks) ----
        from collections import deque

        for _ in qkv_stage(0):
            pass
        g1 = qkv_stage(1)
        feed0 = deque()

        def qkv1_unit():
            next(g1, None)

        feed0.extend([qkv1_unit] * 16)
        y2T[0] = y2pool.tile([128, T], F32R, tag="y2T0", name="y2T0")
        for _ in attention_stage(0, [0, 1, 2, 3], feed0, 1, do_out=False):
            pass
        while feed0:
            feed0.popleft()()
        y2T[1] = y2pool.tile([128, T], F32R, tag="y2T1", name="y2T1")
        feed1 = deque()
        for _ in attention_stage(1, [0, 3, 2, 1], feed1, 2, do_out=True):
            pass

    nc.compile()
    return nc


def _round_fp32r(a):
    """Round-to-nearest-even to fp32r (1s+8e+11m, value kept in the fp32 high bits)."""
    u = np.ascontiguousarray(a, np.float32).view(np.uint32)
    keep = u & np.uint32(0xFFFFF000)
    rem = u & np.uint32(0x00000FFF)
    lsb = (u >> np.uint32(12)) & np.uint32(1)
    up = (rem > 0x800) | ((rem == 0x800) & (lsb == 1))
    return (keep + (up.astype(np.uint32) << np.uint32(12))).view(np.float32)


def _host_inputs(x, attention_mask, qkv_w, qkv_b, out_w):
    """Build device input tensors. Returns per-core list of dicts."""
    import ml_dtypes

    BF = ml_dtypes.bfloat16
    x = np.ascontiguousarray(np.asarray(x, np.float32))
    qkv_w = np.asarray(qkv_w, np.float32)
    qkv_b = np.asarray(qkv_b, np.float32)
    out_w = np.asarray(out_w, np.float32)
    am = np.asarray(attention_mask)

    # xT[p, k, t] = x[b][t, 128k + p]
    xT_b = [
        np.ascontiguousarray(
            x[b].T.reshape(CT, 128, T).transpose(1, 0, 2).astype(BF)
        )
        for b in range(B)
    ]

    # RoPE tables (match reference: interleaved rotate, concatenated freq table)
    inv_freq = 1.0 / (10000.0 ** (np.arange(0, DH, 2, dtype=np.float64) / DH))
    tt = np.arange(T, dtype=np.float64)
    freqs = np.outer(tt, inv_freq)  # [T, 32]
    emb = np.concatenate([freqs, freqs], axis=-1)  # [T, 64]
    cos = np.cos(emb).astype(np.float32).T  # [64, T]
    sin = np.sin(emb).astype(np.float32).T  # [64, T]
    sinTt64 = np.empty((DH, T), np.float32)
    sinTt64[0::2] = sin[1::2]  # sinTt[2i]   = +sin[2i+1]
    sinTt64[1::2] = -sin[0::2]  # sinTt[2i+1] = -sin[2i]
    cosT = np.ascontiguousarray(np.tile(cos, (2, 1)).astype(BF))  # [128, T]
    sinTt = np.ascontiguousarray(np.tile(sinTt64, (2, 1)).astype(BF))

    # tri mask for the diagonal partial block: query 512j+128m+c' vs key
    # 512j+128m+p -> valid iff c' >= p, identical for every m.
    cc = np.arange(128)[None, :]
    pp = np.arange(128)[:, None]
    triC = np.ascontiguousarray((cc >= pp).astype(BF))

    key_ok = am.astype(bool).reshape(B, NTK, 128)  # [b, t, p]
    expb_b = [
        np.ascontiguousarray(np.where(key_ok[b], 0.0, NEG).astype(np.float32).T)
        for b in range(B)
    ]

    per_core = []
    for c in range(NCORES):
        b_c, hg = divmod(c, 4)
        # qkvwT[hp, s, p, k, m] = qkv_w[s*C + r0 + m, 128k + p]
        qkvwT = np.empty((2, 3, 128, CT, 128), np.float32)
        bqkv = np.zeros((2, 128, 4), np.float32)
        vbb = np.empty((2, 128, 128), np.float32)
        owT = np.empty((2, 128, 8, 128), np.float32)
        for hp in range(2):
            r0 = 256 * hg + 128 * hp
            for s in range(3):
                w = qkv_w[s * C + r0 : s * C + r0 + 128, :]  # [rows 128, C]
                # -> [p, k, m]: w.T reshaped (CT, 128, C-part) transposed
                qkvwT[hp, s] = w.T.reshape(CT, 128, 128).transpose(1, 0, 2)
                if s < 2:
                    bqkv[hp, :, s] = qkv_b[s * C + r0 : s * C + r0 + 128]
            vbb[hp] = np.broadcast_to(
                qkv_b[2 * C + r0 : 2 * C + r0 + 128][None, :], (128, 128)
            )
            ow = out_w[:, r0 : r0 + 128]  # [1024, 128]
            # owT[p, mt, m] = out_w[128mt + m, r0 + p]
            owT[hp] = _round_fp32r(ow.reshape(8, 128, 128).transpose(2, 0, 1))
        per_core.append(
            dict(
                xT=xT_b[b_c],
                qkvwT=np.ascontiguousarray(qkvwT.astype(BF)),
                bqkv=bqkv,
                vbb=vbb,
                owT=np.ascontiguousarray(owT),
                cosT=cosT,
                sinTt=sinTt,
                triC=triC,
                expb=expb_b[b_c],
            )
        )
    return per_core


def _gather(results, attention_mask, out_b):
    acc = np.zeros((B, T, C), np.float64)
    for c in range(NCORES):
        part = np.asarray(results[c]["outp"], np.float32)  # [128, 8, T]
        acc[c // 4] += part.transpose(1, 0, 2).reshape(C, T).T
    qm = np.asarray(attention_mask).astype(bool)
    out = np.where(qm[..., None], acc, 0.0) + np.asarray(out_b, np.float64)[None, None]
    return out.astype(np.float32)


def kernel(x, attention_mask, qkv_w, qkv_b, out_w, out_b, _trace=False):
    global LAST_RESULTS
    from concourse.bass_utils import run_bass_kernel_spmd

    key = ("nc", bool(np.any(np.asarray(qkv_b))))
    if key not in _PROGRAM_CACHE:
        _PROGRAM_CACHE[key] = _build_program(has_qkv_bias=key[1])
    nc = _PROGRAM_CACHE[key]

    in_maps = _host_inputs(x, attention_mask, qkv_w, qkv_b, out_w)

    res = run_bass_kernel_spmd(
        nc,
        in_maps,
        core_ids=list(range(NCORES)),
        trace=_trace,
        trace_cores=list(range(NCORES)) if _trace else None,
        stitch_traces=bool(_trace),
    )
    LAST_RESULTS = res
    return _gather(res.results, attention_mask, out_b)
